# Initial kernel scaffold
#
"""ComplexPolarAttention Trainium2 kernel.

score_ij = sum_d mag_i,d mag_j,d cos(phase_i,d - phase_j,d)
         = a_i . a_j + b_i . b_j          with a = mag*cos(phase), b = mag*sin(phase)
out_mag   = softmax(score, axis=1) @ mag
out_phase = softmax(score, axis=1) @ phase

Strategy (8 NeuronCores, SPMD, no collectives):
  - Rows (queries) sharded; keys replicated. Per-core inputs are ROTATED
    along the key axis so that core c's queries are always columns 0..q of
    its own key panel (softmax over keys is permutation invariant), so the
    query operand is just a slice of the key panel.
  - The packed ab^T = [a|b]^T [128=2D, N] bf16 panel fuses the two score
    GEMMs into ONE K=128 matmul per key block of 128 and halves the input
    stream. Scores are computed transposed, S^T[k_blk=128, q] fp32 in PSUM
    (matmul outputs must be fp32), one wide [128, 1024] exp per key block;
    the exp ACTIVATEs are the bottleneck engine (~70us) and everything
    else is sized to hide underneath them. Scores < 88: exp can't overflow.
  - Value matmuls use ONE packed bf16 stationary [mag|phase] (128 cols)
    per key block with es (bf16) as the 2x512 moving operand; numerator
    rows [0:64]=mag^T, [64:128]=phase^T accumulate over all 64 key blocks
    into two [128, 512] fp32 PSUM banks (one per q half).
  - The softmax denominator: DVE keeps a running bf16 es sum (16-bit rate)
    in chains of 10 key blocks, double-buffered; GpSimd (otherwise idle)
    flushes each finished chain into a fp32 master, bounding the bf16
    running-sum rounding to ~0.1%. The last chain flush runs on DVE and
    the last two key blocks feed the final ones^T [128,1] denominator
    matmuls directly, keeping the serial tail ~2us.
  - The final divide happens on host during the gather.
  - All DRAM inputs are chunk-major so every dma_start reads one fully
    contiguous block; the ab^T chunks ride the sync HWDGE queue, the value
    matrix the gpsimd SWDGE queue, so the k-loop's critical first chunk
    lands as early as possible and later chunks stream in under compute.
"""

import numpy as np
from contextlib import ExitStack

import concourse.bass as bass
import concourse.tile as tile
from concourse import bacc, mybir
from concourse.bass_utils import run_bass_kernel_spmd

F32 = mybir.dt.float32
F32R = mybir.dt.float32r
BF16 = mybir.dt.bfloat16

CHAIN = 9           # key blocks per bf16 partial-sum chain
NDIRECT = 3         # trailing key blocks folded straight into psD matmuls


def abt_chunk_widths(n):
    widths, rem = [], n
    for w in (512, 512, 1024):
        if rem >= w:
            widths.append(w)
            rem -= w
    while rem:
        w = min(2048, rem)
        widths.append(w)
        rem -= w
    return widths


def build_program(n=8192, d=64, n_cores=8, enable_asserts=False):
    """Build the SPMD Bass program. Every core runs identical IR; per-core
    behavior comes only from per-core (rotated) input data."""
    assert d == 64
    q = n // n_cores            # queries per core
    kblocks = n // 128          # key blocks of 128
    qblk = q // 2               # half per matmul (psum bank = 512 fp32)
    assert qblk <= 512 and n % 128 == 0 and q <= 1024

    nc = bacc.Bacc(
        "TRN2",
        target_bir_lowering=False,
        debug=False,
        enable_asserts=enable_asserts,
        num_devices=n_cores,
    )

    # ---- DRAM I/O (all per-core arrays rotated so queries = keys[0:q]) ----
    chunks = abt_chunk_widths(n)
    vchunk = max(1, kblocks // 16)
    nvch = kblocks // vchunk
    abt_in = [nc.dram_tensor(f"abt{i}", [128, w], BF16,
                             kind="ExternalInput").ap()
              for i, w in enumerate(chunks)]
    # packed [mag | phase] value matrix, chunk-major [nvch, 128, vchunk*128]
    vt = nc.dram_tensor("vt", [nvch, 128, vchunk * 128], BF16,
                        kind="ExternalInput").ap()
    ones_in = nc.dram_tensor("onesv", [128, 1], F32R,
                             kind="ExternalInput").ap()
    onesb_in = nc.dram_tensor("onesb", [128, 1], BF16,
                              kind="ExternalInput").ap()

    onum = nc.dram_tensor("onum", [128, q], BF16, kind="ExternalOutput").ap()
    oden = nc.dram_tensor("oden", [1, q], F32, kind="ExternalOutput").ap()

    nchain = (kblocks - NDIRECT + CHAIN - 1) // CHAIN

    with tile.TileContext(nc) as tc, ExitStack() as ctx:
        persist = ctx.enter_context(tc.tile_pool(name="persist", bufs=1))
        epool = ctx.enter_context(tc.tile_pool(name="exps", bufs=7))
        opool = ctx.enter_context(tc.tile_pool(name="outs", bufs=4))
        spool = ctx.enter_context(tc.tile_pool(name="scores", bufs=3, space="PSUM"))
        apool = ctx.enter_context(tc.tile_pool(name="accum", bufs=1, space="PSUM"))

        abt = persist.tile([128, n], BF16)       # [a|b]^T for all keys
        vt_t = persist.tile([128, kblocks, 128], BF16)
        ones = persist.tile([128, 1], F32R)
        ones_bf = persist.tile([128, 1], BF16)
        es_sumW = [persist.tile([128, q], BF16, name=f"esW{i}")
                   for i in range(2)]            # double-buffered bf16 chains
        es_sumM = persist.tile([128, q], F32R)   # fp32 master sum

        # ab^T chunks on the sync queue -- chunk 0 gates the first matmul
        off = 0
        for i, w in enumerate(chunks):
            nc.sync.dma_start(out=abt[:, off:off + w], in_=abt_in[i])
            off += w
        abq = abt[:, 0:q]          # queries are the first q key columns

        # value matrix on the gpsimd queue, fine-grained and interleaved
        # so the first key blocks' stationaries land just after exp0
        for vi in range(nvch):
            b0 = vi * vchunk
            b1 = b0 + vchunk
            nc.gpsimd.dma_start(out=vt_t[:, b0:b1, :], in_=vt[vi, :, :])
        nc.sync.dma_start(out=ones[:, :], in_=ones_in)
        nc.sync.dma_start(out=ones_bf[:, :], in_=onesb_in)

        # ---- PSUM budget: scores [128,1024] fp32 x3bufs = 6 banks, accA/B 2.
        accA = apool.tile([128, qblk], F32, name="accA", tag="accA")
        accB = apool.tile([128, qblk], F32, name="accB", tag="accB")

        def chain_op(kb, eh):
            """bf16 running es sum; chains of CHAIN flushed into the fp32
            master (GpSimd, except the last chain -> DVE to shorten the
            tail); the last NDIRECT blocks skip the chain entirely."""
            if kb >= kblocks - NDIRECT:
                return
            c, ci = divmod(kb, CHAIN)
            w = es_sumW[c % 2]
            t1, o1, t2, o2 = eh
            if t1 is t2:
                pieces = [(w[:, :], t1[:, o1:o1 + q])]
            else:
                pieces = [(w[:, 0:qblk], t1[:, o1:o1 + qblk]),
                          (w[:, qblk:q], t2[:, o2:o2 + qblk])]
            for dst, src in pieces:
                if ci == 0:
                    nc.vector.tensor_copy(dst, src)
                else:
                    nc.vector.tensor_add(dst, dst, src)
            if ci == CHAIN - 1 or kb == kblocks - NDIRECT - 1:
                eng = nc.vector if c == nchain - 1 else nc.gpsimd
                if c == 0:
                    eng.tensor_copy(es_sumM[:, :], w[:, :])
                else:
                    eng.tensor_add(es_sumM[:, :], es_sumM[:, :], w[:, :])

        def value_mms(kb, eh):
            first, last = (kb == 0), (kb == kblocks - 1)
            for j in range(2):
                acc = accA if j == 0 else accB
                t, o = eh[2 * j], eh[2 * j + 1]
                nc.tensor.matmul(
                    out=acc[:, :], lhsT=vt_t[:, kb, :],
                    rhs=t[:, o:o + qblk],
                    start=first, stop=last)
            chain_op(kb, eh)

        # the first SPLIT_HEAD blocks exp in 512-col halves right behind
        # their score matmuls: ACT (the bottleneck engine) starts ~1us
        # earlier than if it waited for a full [128,1024] score tile
        SPLIT_HEAD = 2
        es_hist = []
        for kb in range(kblocks):
            if len(es_hist) >= 2:
                value_mms(kb - 2, es_hist[-2])
            ss = spool.tile([128, q], F32)
            halves = []
            for j in range(2):
                qsl = slice(j * qblk, (j + 1) * qblk)
                nc.tensor.matmul(
                    out=ss[:, qsl],
                    lhsT=abt[:, kb * 128:(kb + 1) * 128],
                    rhs=abq[:, qsl],
                    start=True, stop=True,
                )
                if kb < SPLIT_HEAD:
                    e = epool.tile([128, qblk], BF16)
                    nc.scalar.activation(
                        e[:, :], ss[:, qsl],
                        mybir.ActivationFunctionType.Exp)
                    halves.append(e)
            if kb < SPLIT_HEAD:
                es_hist.append((halves[0], 0, halves[1], 0))
            else:
                es = epool.tile([128, q], BF16)
                nc.scalar.activation(
                    es[:, :], ss[:, :], mybir.ActivationFunctionType.Exp,
                )
                es_hist.append((es, 0, es, qblk))
        value_mms(kblocks - 2, es_hist[-2])

        # denominator: psD[j] = ones^T @ (fp32 master + last NDIRECT blocks'
        # es, which skipped the DVE chain). Everything except the very last
        # block's parts is issued before the last value matmuls so it runs
        # in the final exp's shadow.
        psD = []
        for j in range(2):
            qsl = slice(j * qblk, (j + 1) * qblk)
            pd = spool.tile([1, qblk], F32, name=f"psD{j}", tag="ss")
            nc.tensor.matmul(out=pd[:, :], lhsT=ones[:, :],
                             rhs=es_sumM[:, qsl], start=True, stop=False)
            psD.append(pd)
        for kb in range(kblocks - NDIRECT, kblocks - 1):
            for j in range(2):
                t, o = es_hist[kb][2 * j], es_hist[kb][2 * j + 1]
                nc.tensor.matmul(out=psD[j][:, :], lhsT=ones_bf[:, :],
                                 rhs=t[:, o:o + qblk],
                                 start=False, stop=False)
        value_mms(kblocks - 1, es_hist[-1])
        for j in range(2):
            t, o = es_hist[-1][2 * j], es_hist[-1][2 * j + 1]
            nc.tensor.matmul(out=psD[j][:, :], lhsT=ones_bf[:, :],
                             rhs=t[:, o:o + qblk],
                             start=False, stop=True)

        # outputs: PSUM -> SBUF (DVE for half 0, ACT for half 1) -> one DMA
        # per DRAM tensor (each dma_start pays ~0.6us of descriptor gen);
        # the small oD copies go first so the oden DMA launches early
        oD = opool.tile([1, q], F32, tag="oD")
        nc.vector.tensor_copy(oD[:, 0:qblk], psD[0][:, :])
        nc.scalar.activation(oD[:, qblk:q], psD[1][:, :],
                             mybir.ActivationFunctionType.Copy)
        # oden rides the gpsimd queue: its ~0.6us descriptor gen then runs
        # in parallel with onum's on the sync queue
        nc.gpsimd.dma_start(out=oden, in_=oD[:, :])
        oN = opool.tile([128, q], BF16, tag="oN")
        nc.vector.tensor_copy(oN[:, 0:qblk], accA[:, :])
        nc.scalar.activation(oN[:, qblk:q], accB[:, :],
                             mybir.ActivationFunctionType.Copy)
        nc.sync.dma_start(out=onum, in_=oN[:, :])

    nc.compile()
    return nc


def make_inputs(mag, phase, n_cores=8):
    """Host-side sharding/layout prep -> per-core (key-rotated) input maps."""
    import ml_dtypes
    bf16 = ml_dtypes.bfloat16
    n, d = mag.shape
    q = n // n_cores
    kblocks = n // 128
    mag = np.ascontiguousarray(mag, dtype=np.float32)
    phase = np.ascontiguousarray(phase, dtype=np.float32)

    a = mag * np.cos(phase)
    b = mag * np.sin(phase)
    abt_g = np.concatenate([a.T, b.T], axis=0).astype(bf16)   # [128, n]
    v_nat = np.concatenate([mag, phase], axis=1).astype(bf16)  # [n, 128]

    chunks = abt_chunk_widths(n)
    vchunk = max(1, kblocks // 16)
    nvch = kblocks // vchunk

    def tile_nat(x):  # [n, m] -> [nvch, 128, vchunk*m] chunk-major
        m = x.shape[1]
        y = x.reshape(nvch, vchunk, 128, m).transpose(0, 2, 1, 3)
        return np.ascontiguousarray(y.reshape(nvch, 128, vchunk * m))

    in_maps = []
    for c in range(n_cores):
        r = c * q
        abt_c = np.roll(abt_g, -r, axis=1)
        m = {"vt": tile_nat(np.roll(v_nat, -r, axis=0)),
             "onesv": np.ones((128, 1), np.float32),
             "onesb": np.ones((128, 1), bf16)}
        off = 0
        for i, w in enumerate(chunks):
            m[f"abt{i}"] = np.ascontiguousarray(abt_c[:, off:off + w])
            off += w
        in_maps.append(m)
    return in_maps


def gather_outputs(results, n, d, n_cores=8):
    """Per-core [128,q] transposed unnormalized sums + [1,q] denominators
    -> full outputs."""
    new_mag = np.empty((n, d), np.float32)
    new_phase = np.empty((n, d), np.float32)
    q = n // n_cores
    for c in range(n_cores):
        onum = np.asarray(results[c]["onum"]).astype(np.float32)  # [128, q]
        den = np.asarray(results[c]["oden"]).astype(np.float32)   # [1, q]
        qsl = slice(c * q, (c + 1) * q)
        new_mag[qsl] = (onum[:64, :] / den).T
        new_phase[qsl] = (onum[64:128, :] / den).T
    return new_mag, new_phase


_PROGRAM_CACHE = {}


def _get_program(n, d, n_cores):
    key = (n, d, n_cores)
    if key not in _PROGRAM_CACHE:
        _PROGRAM_CACHE[key] = build_program(n=n, d=d, n_cores=n_cores)
    return _PROGRAM_CACHE[key]


def kernel(mag, phase):
    mag = np.asarray(mag, dtype=np.float32)
    phase = np.asarray(phase, dtype=np.float32)
    n, d = mag.shape
    n_cores = 8
    nc = _get_program(n, d, n_cores)
    in_maps = make_inputs(mag, phase, n_cores=n_cores)
    res = run_bass_kernel_spmd(nc, in_maps, list(range(n_cores)))
    return gather_outputs(res.results, n, d, n_cores=n_cores)



# revision 1
# speedup vs baseline: 1.2388x; 1.2388x over previous
"""ComplexPolarAttention Trainium2 kernel.

score_ij = sum_d mag_i,d mag_j,d cos(phase_i,d - phase_j,d)
         = a_i . a_j + b_i . b_j          with a = mag*cos(phase), b = mag*sin(phase)
out_mag   = softmax(score, axis=1) @ mag
out_phase = softmax(score, axis=1) @ phase

Strategy (8 NeuronCores, SPMD, no collectives):
  - Rows (queries) sharded; keys replicated. Per-core inputs are ROTATED
    along the key axis so that core c's queries are always columns 0..q of
    its own key panel (softmax over keys is permutation invariant), so the
    query operand is just a slice of the key panel.
  - The packed ab^T = [a|b]^T [128=2D, N] bf16 panel fuses the two score
    GEMMs into ONE K=128 matmul per key block of 128 and halves the input
    stream. Scores are computed transposed, S^T[k_blk=128, q] fp32 in PSUM
    (matmul outputs must be fp32), one wide [128, 1024] exp per key block;
    the exp ACTIVATEs are the bottleneck engine (~70us) and everything
    else is sized to hide underneath them. Scores < 88: exp can't overflow.
  - Value matmuls use ONE packed bf16 stationary [mag|phase] (128 cols)
    per key block with es (bf16) as the 2x512 moving operand; numerator
    rows [0:64]=mag^T, [64:128]=phase^T accumulate over all 64 key blocks
    into two [128, 512] fp32 PSUM banks (one per q half).
  - The softmax denominator: DVE keeps a running bf16 es sum (16-bit rate)
    in chains of 10 key blocks, double-buffered; GpSimd (otherwise idle)
    flushes each finished chain into a fp32 master, bounding the bf16
    running-sum rounding to ~0.1%. The last chain flush runs on DVE and
    the last two key blocks feed the final ones^T [128,1] denominator
    matmuls directly, keeping the serial tail ~2us.
  - The final divide happens on host during the gather.
  - All DRAM inputs are chunk-major so every dma_start reads one fully
    contiguous block; the ab^T chunks ride the sync HWDGE queue, the value
    matrix the gpsimd SWDGE queue, so the k-loop's critical first chunk
    lands as early as possible and later chunks stream in under compute.
"""

import numpy as np
from contextlib import ExitStack

import concourse.bass as bass
import concourse.tile as tile
from concourse import bacc, mybir
from concourse.bass_utils import run_bass_kernel_spmd

F32 = mybir.dt.float32
F32R = mybir.dt.float32r
BF16 = mybir.dt.bfloat16

CHAIN = 9           # key blocks per bf16 partial-sum chain
NDIRECT = 3         # trailing key blocks folded straight into psD matmuls


def abt_chunk_widths(n):
    widths, rem = [], n
    for w in (512, 512, 1024):
        if rem >= w:
            widths.append(w)
            rem -= w
    while rem:
        w = min(2048, rem)
        widths.append(w)
        rem -= w
    return widths


def build_program(n=8192, d=64, n_cores=8, enable_asserts=False):
    """Build the SPMD Bass program. Every core runs identical IR; per-core
    behavior comes only from per-core (rotated) input data."""
    assert d == 64
    q = n // n_cores            # queries per core
    kblocks = n // 128          # key blocks of 128
    qblk = q // 2               # half per matmul (psum bank = 512 fp32)
    assert qblk <= 512 and n % 128 == 0 and q <= 1024

    nc = bacc.Bacc(
        "TRN2",
        target_bir_lowering=False,
        debug=False,
        enable_asserts=enable_asserts,
        num_devices=n_cores,
    )

    # ---- DRAM I/O (all per-core arrays rotated so queries = keys[0:q]) ----
    chunks = abt_chunk_widths(n)
    vchunk = max(1, kblocks // 16)
    nvch = kblocks // vchunk
    abt_in = [nc.dram_tensor(f"abt{i}", [128, w], BF16,
                             kind="ExternalInput").ap()
              for i, w in enumerate(chunks)]
    # packed [mag | phase] value matrix, chunk-major [nvch, 128, vchunk*128]
    vt = nc.dram_tensor("vt", [nvch, 128, vchunk * 128], BF16,
                        kind="ExternalInput").ap()
    ones_in = nc.dram_tensor("onesv", [128, 1], F32R,
                             kind="ExternalInput").ap()
    onesb_in = nc.dram_tensor("onesb", [128, 1], BF16,
                              kind="ExternalInput").ap()

    onum = nc.dram_tensor("onum", [128, q], BF16, kind="ExternalOutput").ap()
    oden = nc.dram_tensor("oden", [1, q], F32, kind="ExternalOutput").ap()

    nchain = (kblocks - NDIRECT + CHAIN - 1) // CHAIN

    with tile.TileContext(nc) as tc, ExitStack() as ctx:
        persist = ctx.enter_context(tc.tile_pool(name="persist", bufs=1))
        epool = ctx.enter_context(tc.tile_pool(name="exps", bufs=7))
        opool = ctx.enter_context(tc.tile_pool(name="outs", bufs=4))
        spool = ctx.enter_context(tc.tile_pool(name="scores", bufs=3, space="PSUM"))
        apool = ctx.enter_context(tc.tile_pool(name="accum", bufs=1, space="PSUM"))

        abt = persist.tile([128, n], BF16)       # [a|b]^T for all keys
        vt_t = persist.tile([128, kblocks, 128], BF16)
        ones = persist.tile([128, 1], F32R)
        ones_bf = persist.tile([128, 1], BF16)
        es_sumW = [persist.tile([128, q], BF16, name=f"esW{i}")
                   for i in range(2)]            # double-buffered bf16 chains
        es_sumM = persist.tile([128, q], F32R)   # fp32 master sum

        # ab^T chunks on the sync queue -- chunk 0 gates the first matmul
        off = 0
        for i, w in enumerate(chunks):
            nc.sync.dma_start(out=abt[:, off:off + w], in_=abt_in[i])
            off += w
        abq = abt[:, 0:q]          # queries are the first q key columns

        # value matrix on the gpsimd queue, fine-grained and interleaved
        # so the first key blocks' stationaries land just after exp0
        for vi in range(nvch):
            b0 = vi * vchunk
            b1 = b0 + vchunk
            nc.gpsimd.dma_start(out=vt_t[:, b0:b1, :], in_=vt[vi, :, :])
        nc.sync.dma_start(out=ones[:, :], in_=ones_in)
        nc.sync.dma_start(out=ones_bf[:, :], in_=onesb_in)

        # ---- PSUM budget: scores [128,1024] fp32 x3bufs = 6 banks, accA/B 2.
        accA = apool.tile([128, qblk], F32, name="accA", tag="accA")
        accB = apool.tile([128, qblk], F32, name="accB", tag="accB")

        def chain_op(kb, eh):
            """bf16 running es sum; chains of CHAIN flushed into the fp32
            master (GpSimd, except the last chain -> DVE to shorten the
            tail); the last NDIRECT blocks skip the chain entirely."""
            if kb >= kblocks - NDIRECT:
                return
            c, ci = divmod(kb, CHAIN)
            w = es_sumW[c % 2]
            t1, o1, t2, o2 = eh
            if t1 is t2:
                pieces = [(w[:, :], t1[:, o1:o1 + q])]
            else:
                pieces = [(w[:, 0:qblk], t1[:, o1:o1 + qblk]),
                          (w[:, qblk:q], t2[:, o2:o2 + qblk])]
            for dst, src in pieces:
                if ci == 0:
                    nc.vector.tensor_copy(dst, src)
                else:
                    nc.vector.tensor_add(dst, dst, src)
            if ci == CHAIN - 1 or kb == kblocks - NDIRECT - 1:
                eng = nc.vector if c == nchain - 1 else nc.gpsimd
                if c == 0:
                    eng.tensor_copy(es_sumM[:, :], w[:, :])
                else:
                    eng.tensor_add(es_sumM[:, :], es_sumM[:, :], w[:, :])

        def value_mms(kb, eh):
            first, last = (kb == 0), (kb == kblocks - 1)
            for j in range(2):
                acc = accA if j == 0 else accB
                t, o = eh[2 * j], eh[2 * j + 1]
                nc.tensor.matmul(
                    out=acc[:, :], lhsT=vt_t[:, kb, :],
                    rhs=t[:, o:o + qblk],
                    start=first, stop=last)
            chain_op(kb, eh)

        # the first SPLIT_HEAD blocks exp in 512-col halves right behind
        # their score matmuls: ACT (the bottleneck engine) starts ~1us
        # earlier than if it waited for a full [128,1024] score tile
        SPLIT_HEAD = 2
        es_hist = []
        for kb in range(kblocks):
            if len(es_hist) >= 2:
                value_mms(kb - 2, es_hist[-2])
            ss = spool.tile([128, q], F32)
            halves = []
            for j in range(2):
                qsl = slice(j * qblk, (j + 1) * qblk)
                nc.tensor.matmul(
                    out=ss[:, qsl],
                    lhsT=abt[:, kb * 128:(kb + 1) * 128],
                    rhs=abq[:, qsl],
                    start=True, stop=True,
                )
                if kb < SPLIT_HEAD:
                    e = epool.tile([128, qblk], BF16)
                    nc.scalar.activation(
                        e[:, :], ss[:, qsl],
                        mybir.ActivationFunctionType.Exp)
                    halves.append(e)
            if kb < SPLIT_HEAD:
                es_hist.append((halves[0], 0, halves[1], 0))
            else:
                es = epool.tile([128, q], BF16)
                nc.scalar.activation(
                    es[:, :], ss[:, :], mybir.ActivationFunctionType.Exp,
                )
                es_hist.append((es, 0, es, qblk))
        value_mms(kblocks - 2, es_hist[-2])

        # denominator: psD[j] = ones^T @ (fp32 master + last NDIRECT blocks'
        # es, which skipped the DVE chain). Everything except the very last
        # block's parts is issued before the last value matmuls so it runs
        # in the final exp's shadow.
        psD = []
        for j in range(2):
            qsl = slice(j * qblk, (j + 1) * qblk)
            pd = spool.tile([1, qblk], F32, name=f"psD{j}", tag="ss")
            nc.tensor.matmul(out=pd[:, :], lhsT=ones[:, :],
                             rhs=es_sumM[:, qsl], start=True, stop=False)
            psD.append(pd)
        for kb in range(kblocks - NDIRECT, kblocks - 1):
            for j in range(2):
                t, o = es_hist[kb][2 * j], es_hist[kb][2 * j + 1]
                nc.tensor.matmul(out=psD[j][:, :], lhsT=ones_bf[:, :],
                                 rhs=t[:, o:o + qblk],
                                 start=False, stop=False)
        value_mms(kblocks - 1, es_hist[-1])
        for j in range(2):
            t, o = es_hist[-1][2 * j], es_hist[-1][2 * j + 1]
            nc.tensor.matmul(out=psD[j][:, :], lhsT=ones_bf[:, :],
                             rhs=t[:, o:o + qblk],
                             start=False, stop=True)

        # outputs: PSUM -> SBUF (DVE for half 0, ACT for half 1) -> one DMA
        # per DRAM tensor (each dma_start pays ~0.6us of descriptor gen);
        # the small oD copies go first so the oden DMA launches early
        oD = opool.tile([1, q], F32, tag="oD")
        nc.vector.tensor_copy(oD[:, 0:qblk], psD[0][:, :])
        nc.scalar.activation(oD[:, qblk:q], psD[1][:, :],
                             mybir.ActivationFunctionType.Copy)
        # oden rides the gpsimd queue: its ~0.6us descriptor gen then runs
        # in parallel with onum's on the sync queue
        nc.gpsimd.dma_start(out=oden, in_=oD[:, :])
        oN = opool.tile([128, q], BF16, tag="oN")
        nc.vector.tensor_copy(oN[:, 0:qblk], accA[:, :])
        nc.scalar.activation(oN[:, qblk:q], accB[:, :],
                             mybir.ActivationFunctionType.Copy)
        nc.sync.dma_start(out=onum, in_=oN[:, :])

    nc.compile()
    return nc


def make_inputs(mag, phase, n_cores=8):
    """Host-side sharding/layout prep -> per-core (key-rotated) input maps."""
    import ml_dtypes
    bf16 = ml_dtypes.bfloat16
    n, d = mag.shape
    q = n // n_cores
    kblocks = n // 128
    mag = np.ascontiguousarray(mag, dtype=np.float32)
    phase = np.ascontiguousarray(phase, dtype=np.float32)

    a = mag * np.cos(phase)
    b = mag * np.sin(phase)
    abt_g = np.concatenate([a.T, b.T], axis=0).astype(bf16)   # [128, n]
    v_nat = np.concatenate([mag, phase], axis=1).astype(bf16)  # [n, 128]

    chunks = abt_chunk_widths(n)
    vchunk = max(1, kblocks // 16)
    nvch = kblocks // vchunk

    def tile_nat(x):  # [n, m] -> [nvch, 128, vchunk*m] chunk-major
        m = x.shape[1]
        y = x.reshape(nvch, vchunk, 128, m).transpose(0, 2, 1, 3)
        return np.ascontiguousarray(y.reshape(nvch, 128, vchunk * m))

    in_maps = []
    for c in range(n_cores):
        r = c * q
        abt_c = np.roll(abt_g, -r, axis=1)
        m = {"vt": tile_nat(np.roll(v_nat, -r, axis=0)),
             "onesv": np.ones((128, 1), np.float32),
             "onesb": np.ones((128, 1), bf16)}
        off = 0
        for i, w in enumerate(chunks):
            m[f"abt{i}"] = np.ascontiguousarray(abt_c[:, off:off + w])
            off += w
        in_maps.append(m)
    return in_maps


def gather_outputs(results, n, d, n_cores=8):
    """Per-core [128,q] transposed unnormalized sums + [1,q] denominators
    -> full outputs."""
    new_mag = np.empty((n, d), np.float32)
    new_phase = np.empty((n, d), np.float32)
    q = n // n_cores
    for c in range(n_cores):
        onum = np.asarray(results[c]["onum"]).astype(np.float32)  # [128, q]
        den = np.asarray(results[c]["oden"]).astype(np.float32)   # [1, q]
        qsl = slice(c * q, (c + 1) * q)
        new_mag[qsl] = (onum[:64, :] / den).T
        new_phase[qsl] = (onum[64:128, :] / den).T
    return new_mag, new_phase


_PROGRAM_CACHE = {}


def _get_program(n, d, n_cores):
    key = (n, d, n_cores)
    if key not in _PROGRAM_CACHE:
        _PROGRAM_CACHE[key] = build_program(n=n, d=d, n_cores=n_cores)
    return _PROGRAM_CACHE[key]


def kernel(mag, phase):
    mag = np.asarray(mag, dtype=np.float32)
    phase = np.asarray(phase, dtype=np.float32)
    n, d = mag.shape
    n_cores = 8
    nc = _get_program(n, d, n_cores)
    in_maps = make_inputs(mag, phase, n_cores=n_cores)
    res = run_bass_kernel_spmd(nc, in_maps, list(range(n_cores)))
    return gather_outputs(res.results, n, d, n_cores=n_cores)

